# revision 17
# baseline (speedup 1.0000x reference)
"""Trainium2 Bass kernel for nn_Attention_79645873537262.

Dense attention with per-head bias, key masking, sigmoid gate:
  t = x @ w_proj.T; per head: q,k,v
  a = softmax(scale*q@k.T + bias + mask); y = a@v
  y = sigmoid(x@w_g.T + b_g) * y;  out = y @ w_o.T + b_o

Sharding: tensor-parallel over heads, 2 heads per core on 8 cores.
Each core runs a fully independent program (no collectives): it computes
its 2 heads' attention plus its 128-column slice of the gate, and a
partial o_proj (contribution of its 128 y-columns to all 1024 outputs).
The host sums the 8 partial outputs and adds b_o (the "all-reduce").

On-device layout is transposed ("scores.T" flash style):
  scores.T[k,q] = kT.T@qT in PSUM (q pre-scaled on host);
  p = exp(qk) * eb where eb = exp(bias + mask) is precomputed on the
  host (exp factorizes), streamed bf16, applied with a DVE multiply.
  y.T ext = [v | ones].T @ p gives y.T rows 0..63 and the softmax
  denominator in row 64.

Normalization: the two per-head denominator rows are DMAd into a [2,512]
tile; one K=2 matmul against a constant selector (rows 0..63 <- den_h0,
64..127 <- den_h1) broadcasts them across partitions; one reciprocal and
one [128,512] multiply normalize both heads -- no DRAM round-trip.

All matmuls bf16 (tolerance 2e-2, we land ~5e-3). The last attention
pass is split per q-tile so its normalization + o_proj overlap the
remaining attention compute; the first pass's eb tiles are prefetched
ahead of non-critical weight loads.
"""
import sys
import numpy as np
import ml_dtypes

try:
    import concourse.bass as bass
except ImportError:
    sys.path.insert(0, "/opt/trn_rl_repo")
    import concourse.bass as bass

import concourse.tile as tile
from concourse import bacc, mybir
from concourse.bass_utils import run_bass_kernel_spmd

B, L, E, H = 1, 2048, 1024, 16
HW = E // H                # 64
SCALE = HW ** -0.5
N_CORES = 8
HPC = H // N_CORES         # 2 heads per core
C2 = HPC * HW              # 128 y-columns per core
MASK_NEG = -60.0           # exp(-60 + max_bias) ~ 1e-23: dead keys vanish

f32 = mybir.dt.float32
f32r = mybir.dt.float32r
bf16 = mybir.dt.bfloat16

NE = E // 128              # 8 contraction chunks
NQ = L // 512              # 4 q-tiles of 512
NKT = L // 128             # 16 k-chunks of 128

_compiled = [None]


def _build():
    nc = bacc.Bacc("TRN2", target_bir_lowering=False, debug=False,
                   num_devices=N_CORES)

    xT_ap = nc.dram_tensor("xT", [E, L], bf16, kind="ExternalInput").ap()
    wpT_ap = nc.dram_tensor("wpT", [E, 3 * C2], bf16, kind="ExternalInput").ap()
    ebT_ap = nc.dram_tensor("ebT", [HPC, L, L], bf16, kind="ExternalInput").ap()
    wgT_ap = nc.dram_tensor("wgT", [E, C2], bf16, kind="ExternalInput").ap()
    bgv_ap = nc.dram_tensor("bgv", [C2, 1], f32, kind="ExternalInput").ap()
    woT_ap = nc.dram_tensor("woT", [C2, E], bf16, kind="ExternalInput").ap()
    onescols_ap = nc.dram_tensor("onescols", [128, NKT], bf16, kind="ExternalInput").ap()
    ident_ap = nc.dram_tensor("ident", [128, 128], bf16, kind="ExternalInput").ap()
    sel2_ap = nc.dram_tensor("sel2", [2, 128], f32r, kind="ExternalInput").ap()
    outT_ap = nc.dram_tensor("outT", [E, L], f32, kind="ExternalOutput").ap()

    with tile.TileContext(nc) as tc:
        from contextlib import ExitStack
        with ExitStack() as ctx:
            pers = ctx.enter_context(tc.tile_pool(name="pers", bufs=1))
            work = ctx.enter_context(tc.tile_pool(name="work", bufs=1))
            biasp = ctx.enter_context(tc.tile_pool(name="bias", bufs=6))
            ep = ctx.enter_context(tc.tile_pool(name="ep", bufs=4))
            pp = ctx.enter_context(tc.tile_pool(name="pp", bufs=5))
            nrm = ctx.enter_context(tc.tile_pool(name="nrm", bufs=2))
            outp = ctx.enter_context(tc.tile_pool(name="outp", bufs=3))
            # one PSUM layout for the whole kernel: no pool-transition barrier
            sp = ctx.enter_context(tc.tile_pool(name="s", bufs=3, space="PSUM"))
            yp = ctx.enter_context(tc.tile_pool(name="y", bufs=1, space="PSUM"))

            # --- proj-critical DMAs first (dispatch order matters) ---
            xT_sb = [pers.tile([128, L], bf16, name=f"xT{e}", tag=f"xT{e}")
                     for e in range(NE)]
            wpT_sb = [pers.tile([128, 3 * C2], bf16, name=f"wpT{e}", tag=f"wpT{e}")
                      for e in range(NE)]
            # HAM warm-up: dependency-free matmuls keep the PE issue stream
            # dense while the proj inputs stream in, so the clock un-gates
            # (4/8 -> 8/8) before real work starts instead of ~20us in.
            # memset on DVE (its queue is empty; gpsimd's is full of DMAs).
            warm_sb = pers.tile([128, 512], bf16, tag="warm")
            nc.vector.memset(warm_sb, 0)
            for i in range(14):
                wps = sp.tile([128, 512], f32, name=f"warm{i}", tag="s")
                nc.tensor.matmul(wps, warm_sb[:, 0:128], warm_sb,
                                 start=True, stop=True)

            # startup loads: even chunks on sync, odd on gpsimd (dispatch is
            # ~0.65us serial per DMA per queue) so all proj chunks land ~13us
            for e in range(NE):
                q = nc.sync if e % 2 == 0 else nc.gpsimd
                q.dma_start(wpT_sb[e], wpT_ap[e * 128:(e + 1) * 128, :])
                q.dma_start(xT_sb[e][:, 0:1024],
                            xT_ap[e * 128:(e + 1) * 128, 0:1024])
            for e in range(NE):
                q = nc.sync if e % 2 == 0 else nc.gpsimd
                q.dma_start(xT_sb[e][:, 1024:2048],
                            xT_ap[e * 128:(e + 1) * 128, 1024:2048])
            bgv_sb = pers.tile([C2, 1], f32, tag="bgv")
            nc.sync.dma_start(bgv_sb, bgv_ap)

            # prefetch first attention pass's eb tiles (kt 0..5) before the
            # non-critical loads; exactly `bufs` allocations so no DMA in the
            # queue waits on a consumer (would deadlock wgT behind it).
            preb = []
            for kt in range(6):
                eb_t = biasp.tile([128, 1024], bf16, name=f"ebpre{kt}", tag="bias")
                dma_eng = nc.gpsimd if kt % 2 == 0 else nc.sync
                dma_eng.dma_start(eb_t, ebT_ap[0, kt * 128:(kt + 1) * 128, 0:1024])
                preb.append(eb_t)

            wgT_sb = [pers.tile([128, C2], bf16, name=f"wgT{e}", tag=f"wgT{e}")
                      for e in range(NE)]
            for e in range(NE):
                q = nc.sync if e % 2 == 0 else nc.gpsimd
                q.dma_start(wgT_sb[e], wgT_ap[e * 128:(e + 1) * 128, :])
            ident_sb = pers.tile([128, 128], bf16, tag="ident")
            nc.sync.dma_start(ident_sb, ident_ap)
            sel2_sb = pers.tile([2, 128], f32r, tag="sel2")
            nc.sync.dma_start(sel2_sb, sel2_ap)
            # v tiles: [128 l, 130] per k-chunk: [v_h0 | ones | v_h1 | ones]
            v_all = pers.tile([128, NKT, 130], bf16, tag="v_all")
            nc.sync.dma_start(v_all[:, :, 64:65], onescols_ap.unsqueeze(2))
            nc.sync.dma_start(v_all[:, :, 129:130], onescols_ap.unsqueeze(2))
            woT_sb = pers.tile([C2, E], bf16, tag="woT")
            nc.sync.dma_start(woT_sb, woT_ap)

            q01 = pers.tile([128, L], bf16, tag="q01")
            k01 = pers.tile([128, L], bf16, tag="k01")
            g_sb = pers.tile([128, L], bf16, tag="g")
            g_lo = pers.tile([64, L], bf16, tag="g_lo")
            ygT = pers.tile([128, L], bf16, tag="ygT")
            den_sb = [pers.tile([2, 512], f32r, name=f"den{qt}", tag=f"den{qt}")
                      for qt in range(NQ)]

            # ---------------- proj ----------------
            vT01 = work.tile([128, L], bf16, tag="vT01")
            dests = [q01, k01, vT01]
            for lh in range(2):
                for f in range(3):
                    ps = sp.tile([128, 1024], f32, name=f"pj{f}_{lh}", tag="s")
                    for e in range(NE):
                        for ltq in range(2):
                            nc.tensor.matmul(
                                ps[:, ltq * 512:(ltq + 1) * 512],
                                wpT_sb[e][:, f * 128:(f + 1) * 128],
                                xT_sb[e][:, lh * 1024 + ltq * 512:
                                          lh * 1024 + (ltq + 1) * 512],
                                start=(e == 0), stop=(e == NE - 1))
                    if f == 2:
                        # vT drain on ScalarE (idle until attention exps)
                        nc.scalar.copy(
                            dests[f][:, lh * 1024:(lh + 1) * 1024], ps)
                    else:
                        nc.vector.tensor_copy(
                            dests[f][:, lh * 1024:(lh + 1) * 1024], ps)

            # transpose vT01 -> v_all[:, kt, :] via PE (bf16 transpose)
            for kt in range(NKT):
                ps = sp.tile([128, 128], bf16, name=f"tr{kt}", tag="s")
                nc.tensor.transpose(
                    ps, vT01[:, kt * 128:(kt + 1) * 128], ident_sb)
                nc.vector.tensor_copy(v_all[:, kt, 0:64], ps[:, 0:64])
                nc.vector.tensor_copy(v_all[:, kt, 65:129], ps[:, 64:128])

            # gate: g = sigmoid(wgT.T @ xT + bg)
            for lh in range(2):
                ps = sp.tile([C2, 1024], f32, name=f"pg{lh}", tag="s")
                for e in range(NE):
                    for ltq in range(2):
                        nc.tensor.matmul(
                            ps[:, ltq * 512:(ltq + 1) * 512], wgT_sb[e],
                            xT_sb[e][:, lh * 1024 + ltq * 512:
                                      lh * 1024 + (ltq + 1) * 512],
                            start=(e == 0), stop=(e == NE - 1))
                nc.scalar.activation(
                    g_sb[:, lh * 1024:(lh + 1) * 1024], ps,
                    mybir.ActivationFunctionType.Sigmoid,
                    bias=bgv_sb, scale=1.0)
            # partition-shifted copy of g rows 64..127 for the h1 epilogue mul
            nc.sync.dma_start(g_lo, g_sb[64:128, :])

            # ---------------- attention ----------------
            # Pass over (head, q-tiles), software-pipelined 2 deep: y(kt-2)
            # is emitted after s(kt) so no PE instruction waits at the queue
            # head (queue-head waits read as idle to HAM and latch the clock
            # cold).  hooks[kt] emits the PREVIOUS pass's epilogue / tails
            # inside this pass's kt loop so the PE never goes quiet at a
            # pass boundary.  Each pass returns its epilogue as a closure.
            def attention_pass(h, qts, preb=None, hooks=None):
                hb = h * 64
                q0 = qts[0] * 512
                W = 512 * len(qts)
                y_ps = {qt: yp.tile([65, 512], f32, name=f"y{h}_{qt}",
                                    tag=f"y{qt % 2}", bufs=1) for qt in qts}
                pend = []  # [(kt, p_tile), ...] awaiting y matmuls
                def flush_y(upto):
                    while pend and pend[0][0] <= upto:
                        pkt, pt = pend.pop(0)
                        for i, qt in enumerate(qts):
                            nc.tensor.matmul(
                                y_ps[qt],
                                v_all[:, pkt, h * 65:(h + 1) * 65],
                                pt[:, i * 512:(i + 1) * 512],
                                start=(pkt == 0), stop=(pkt == NKT - 1))
                for kt in range(NKT):
                    if preb is not None and kt < len(preb):
                        eb_t = preb[kt]
                    else:
                        eb_t = biasp.tile([128, W], bf16,
                                          name=f"eb{h}_{q0}_{kt}", tag="bias")
                        dma_eng = nc.gpsimd if kt % 2 == 0 else nc.sync
                        dma_eng.dma_start(
                            eb_t, ebT_ap[h, kt * 128:(kt + 1) * 128,
                                         q0:q0 + W])
                    s_ps = sp.tile([128, W], f32,
                                   name=f"s{h}_{q0}_{kt}", tag="s")
                    for i, qt in enumerate(qts):
                        nc.tensor.matmul(
                            s_ps[:, i * 512:(i + 1) * 512],
                            k01[hb:hb + 64, kt * 128:(kt + 1) * 128],
                            q01[hb:hb + 64, qt * 512:(qt + 1) * 512],
                            start=True, stop=True)
                    flush_y(kt - 2)
                    if hooks and kt in hooks:
                        hooks[kt]()
                    eqk_t = ep.tile([128, W], bf16,
                                    name=f"eqk{h}_{q0}_{kt}", tag="eqk")
                    nc.scalar.activation(
                        eqk_t, s_ps, mybir.ActivationFunctionType.Exp)
                    p_t = pp.tile([128, W], bf16,
                                  name=f"p{h}_{q0}_{kt}", tag="p")
                    nc.vector.tensor_mul(p_t, eqk_t, eb_t)
                    pend.append((kt, p_t))
                flush_y(NKT - 1)

                def epilogue():
                    for qt in qts:
                        qsl = slice(qt * 512, (qt + 1) * 512)
                        denrow = nrm.tile([65, 512], f32r,
                                          name=f"denrow{h}_{qt}", tag="denrow")
                        nc.vector.tensor_copy(denrow[64:65, :],
                                              y_ps[qt][64:65, :])
                        nc.sync.dma_start(den_sb[qt][h:h + 1, :],
                                          denrow[64:65, :])
                        if h == 0:
                            nc.vector.tensor_mul(
                                ygT[0:64, qsl], y_ps[qt][0:64, :],
                                g_sb[0:64, qsl])
                        else:
                            yg1 = nrm.tile([64, 512], bf16,
                                           name=f"yg1_{qt}", tag="yg1")
                            nc.vector.tensor_mul(
                                yg1, y_ps[qt][0:64, :], g_lo[:, qsl])
                            nc.sync.dma_start(ygT[64:128, qsl], yg1)
                return epilogue

            def tail_rb(qt):
                # broadcast both heads' denominators and normalize ygT
                qsl = slice(qt * 512, (qt + 1) * 512)
                ps_rb = sp.tile([128, 512], f32, name=f"rb_ps{qt}", tag="s")
                nc.tensor.matmul(ps_rb, sel2_sb, den_sb[qt],
                                 start=True, stop=True)
                rbT = nrm.tile([128, 512], f32, name=f"rbT{qt}", tag="rbT")
                nc.vector.reciprocal_approx_fast(rbT, ps_rb)
                nc.vector.tensor_mul(ygT[:, qsl], ygT[:, qsl], rbT)

            def tail_oproj(qt):
                qsl = slice(qt * 512, (qt + 1) * 512)
                for eo in range(NE):
                    ps = sp.tile([128, 512], f32, name=f"po{qt}_{eo}", tag="s")
                    nc.tensor.matmul(
                        ps, woT_sb[:, eo * 128:(eo + 1) * 128], ygT[:, qsl],
                        start=True, stop=True)
                    ot = outp.tile([128, 512], f32, name=f"ot{qt}_{eo}", tag="ot")
                    if eo % 2 == 0:
                        nc.vector.tensor_copy(ot, ps)
                    else:
                        nc.scalar.copy(ot, ps)
                    nc.sync.dma_start(
                        outT_ap[eo * 128:(eo + 1) * 128, qsl], ot)

            e0 = attention_pass(0, [0, 1], preb=preb)
            e1 = attention_pass(1, [0, 1], hooks={1: e0})
            e2 = attention_pass(0, [2, 3], hooks={1: e1})
            e3 = attention_pass(1, [2, 3], hooks={
                1: e2,
                3: lambda: tail_rb(0),
                6: lambda: tail_oproj(0),
                9: lambda: tail_rb(1),
                12: lambda: tail_oproj(1),
            })
            e3()
            tail_rb(2)
            tail_oproj(2)
            tail_rb(3)
            tail_oproj(3)

    nc.compile()
    return nc


def kernel(x, mask, bias, w_proj, w_o, b_o, w_g, b_g):
    x = np.asarray(x, dtype=np.float32)
    mask = np.asarray(mask)
    bias = np.asarray(bias, dtype=np.float32)
    w_proj = np.asarray(w_proj, dtype=np.float32)
    w_o = np.asarray(w_o, dtype=np.float32)
    b_o = np.asarray(b_o, dtype=np.float32)
    w_g = np.asarray(w_g, dtype=np.float32)
    b_g = np.asarray(b_g, dtype=np.float32)

    if _compiled[0] is None:
        _compiled[0] = _build()
    nc = _compiled[0]

    bf = ml_dtypes.bfloat16
    xT = np.ascontiguousarray(x[0].T).astype(bf)           # [E, L]
    mask_add = np.where(mask[0], 0.0, MASK_NEG).astype(np.float32)  # [L]
    onescols = np.ones((128, NKT), dtype=bf)
    ident = np.eye(128, dtype=bf)
    sel2 = np.zeros((2, 128), dtype=np.float32)
    sel2[0, 0:64] = 1.0
    sel2[1, 64:128] = 1.0

    in_maps = []
    for c in range(N_CORES):
        heads = [c * HPC + i for i in range(HPC)]
        wpT = np.empty((E, 3 * C2), dtype=np.float32)
        for i, h in enumerate(heads):
            r0 = h * 3 * HW
            wpT[:, 0 * C2 + i * HW: 0 * C2 + (i + 1) * HW] = \
                w_proj[r0: r0 + HW].T * SCALE               # q, pre-scaled
            wpT[:, 1 * C2 + i * HW: 1 * C2 + (i + 1) * HW] = \
                w_proj[r0 + HW: r0 + 2 * HW].T              # k
            wpT[:, 2 * C2 + i * HW: 2 * C2 + (i + 1) * HW] = \
                w_proj[r0 + 2 * HW: r0 + 3 * HW].T          # v
        ebT = np.ascontiguousarray(
            bias[0, :, :, heads].transpose(0, 2, 1))        # [2, Lk, Lq]
        ebT += mask_add[None, :, None]
        ebT = np.exp(ebT).astype(bf)
        cols = slice(c * C2, (c + 1) * C2)
        wgT = np.ascontiguousarray(w_g[cols, :].T).astype(bf)  # [E, C2]
        bgv = np.ascontiguousarray(b_g[cols, None])         # [C2, 1]
        woT = np.ascontiguousarray(w_o[:, cols].T).astype(bf)  # [C2, E]
        in_maps.append({
            "xT": xT, "wpT": wpT.astype(bf), "ebT": ebT, "wgT": wgT,
            "bgv": bgv, "woT": woT, "onescols": onescols, "ident": ident,
            "sel2": sel2,
        })

    res = run_bass_kernel_spmd(nc, in_maps, list(range(N_CORES)))
    acc = res.results[0]["outT"].astype(np.float64)
    for c in range(1, N_CORES):
        acc += res.results[c]["outT"]
    out = acc.T.astype(np.float32) + b_o[None, :]
    return out[None]  # [B, L, E]


# revision 19
# speedup vs baseline: 1.0134x; 1.0134x over previous
"""Trainium2 Bass kernel for nn_Attention_79645873537262.

Dense attention with per-head bias, key masking, sigmoid gate:
  t = x @ w_proj.T; per head: q,k,v
  a = softmax(scale*q@k.T + bias + mask); y = a@v
  y = sigmoid(x@w_g.T + b_g) * y;  out = y @ w_o.T + b_o

Sharding: tensor-parallel over heads, 2 heads per core on 8 cores.
Each core runs a fully independent program (no collectives): it computes
its 2 heads' attention plus its 128-column slice of the gate, and a
partial o_proj (contribution of its 128 y-columns to all 1024 outputs).
The host sums the 8 partial outputs and adds b_o (the "all-reduce").

On-device layout is transposed ("scores.T" flash style):
  scores.T[k,q] = kT.T@qT in PSUM (q pre-scaled on host);
  p = exp(qk) * eb where eb = exp(bias + mask) is precomputed on the
  host (exp factorizes), streamed bf16, applied with a DVE multiply.
  y.T ext = [v | ones].T @ p gives y.T rows 0..63 and the softmax
  denominator in row 64.

Normalization: the two per-head denominator rows are DMAd into a [2,512]
tile; one K=2 matmul against a constant selector (rows 0..63 <- den_h0,
64..127 <- den_h1) broadcasts them across partitions; one reciprocal and
one [128,512] multiply normalize both heads -- no DRAM round-trip.

All matmuls bf16 (tolerance 2e-2, we land ~5e-3). The last attention
pass is split per q-tile so its normalization + o_proj overlap the
remaining attention compute; the first pass's eb tiles are prefetched
ahead of non-critical weight loads.
"""
import sys
import numpy as np
import ml_dtypes

try:
    import concourse.bass as bass
except ImportError:
    sys.path.insert(0, "/opt/trn_rl_repo")
    import concourse.bass as bass

import concourse.tile as tile
from concourse import bacc, mybir
from concourse.bass_utils import run_bass_kernel_spmd

B, L, E, H = 1, 2048, 1024, 16
HW = E // H                # 64
SCALE = HW ** -0.5
N_CORES = 8
HPC = H // N_CORES         # 2 heads per core
C2 = HPC * HW              # 128 y-columns per core
MASK_NEG = -60.0           # exp(-60 + max_bias) ~ 1e-23: dead keys vanish

f32 = mybir.dt.float32
f32r = mybir.dt.float32r
bf16 = mybir.dt.bfloat16

NE = E // 128              # 8 contraction chunks
NQ = L // 512              # 4 q-tiles of 512
NKT = L // 128             # 16 k-chunks of 128

_compiled = [None]


def _build():
    nc = bacc.Bacc("TRN2", target_bir_lowering=False, debug=False,
                   num_devices=N_CORES)

    xT_ap = nc.dram_tensor("xT", [E, L], bf16, kind="ExternalInput").ap()
    wpT_ap = nc.dram_tensor("wpT", [E, 3 * C2], bf16, kind="ExternalInput").ap()
    ebT_ap = nc.dram_tensor("ebT", [HPC, L, L], bf16, kind="ExternalInput").ap()
    wgT_ap = nc.dram_tensor("wgT", [E, C2], bf16, kind="ExternalInput").ap()
    bgv_ap = nc.dram_tensor("bgv", [C2, 1], f32, kind="ExternalInput").ap()
    woT_ap = nc.dram_tensor("woT", [C2, E], bf16, kind="ExternalInput").ap()
    onescols_ap = nc.dram_tensor("onescols", [128, NKT], bf16, kind="ExternalInput").ap()
    ident_ap = nc.dram_tensor("ident", [128, 128], bf16, kind="ExternalInput").ap()
    sel2_ap = nc.dram_tensor("sel2", [2, 128], f32r, kind="ExternalInput").ap()
    outT_ap = nc.dram_tensor("outT", [E, L], f32, kind="ExternalOutput").ap()

    with tile.TileContext(nc) as tc:
        from contextlib import ExitStack
        with ExitStack() as ctx:
            pers = ctx.enter_context(tc.tile_pool(name="pers", bufs=1))
            work = ctx.enter_context(tc.tile_pool(name="work", bufs=1))
            biasp = ctx.enter_context(tc.tile_pool(name="bias", bufs=6))
            ep = ctx.enter_context(tc.tile_pool(name="ep", bufs=4))
            pp = ctx.enter_context(tc.tile_pool(name="pp", bufs=5))
            nrm = ctx.enter_context(tc.tile_pool(name="nrm", bufs=2))
            outp = ctx.enter_context(tc.tile_pool(name="outp", bufs=3))
            # one PSUM layout for the whole kernel: no pool-transition barrier
            sp = ctx.enter_context(tc.tile_pool(name="s", bufs=3, space="PSUM"))
            yp = ctx.enter_context(tc.tile_pool(name="y", bufs=1, space="PSUM"))

            # --- proj-critical DMAs first (dispatch order matters) ---
            xT_sb = [pers.tile([128, L], bf16, name=f"xT{e}", tag=f"xT{e}")
                     for e in range(NE)]
            wpT_sb = [pers.tile([128, 3 * C2], bf16, name=f"wpT{e}", tag=f"wpT{e}")
                      for e in range(NE)]
            # HAM warm-up: dependency-free matmuls keep the PE issue stream
            # dense while the proj inputs stream in, so the clock un-gates
            # (4/8 -> 8/8) before real work starts instead of ~20us in.
            # memset on DVE (its queue is empty; gpsimd's is full of DMAs).
            warm_sb = pers.tile([128, 512], bf16, tag="warm")
            nc.vector.memset(warm_sb, 0)
            for i in range(14):
                wps = sp.tile([128, 512], f32, name=f"warm{i}", tag="s")
                nc.tensor.matmul(wps, warm_sb[:, 0:128], warm_sb,
                                 start=True, stop=True)

            # startup loads: even chunks on sync, odd on gpsimd (dispatch is
            # ~0.65us serial per DMA per queue) so all proj chunks land ~13us
            for e in range(NE):
                q = nc.sync if e % 2 == 0 else nc.gpsimd
                q.dma_start(wpT_sb[e], wpT_ap[e * 128:(e + 1) * 128, :])
                q.dma_start(xT_sb[e][:, 0:1024],
                            xT_ap[e * 128:(e + 1) * 128, 0:1024])
            for e in range(NE):
                q = nc.sync if e % 2 == 0 else nc.gpsimd
                q.dma_start(xT_sb[e][:, 1024:2048],
                            xT_ap[e * 128:(e + 1) * 128, 1024:2048])
            bgv_sb = pers.tile([C2, 1], f32, tag="bgv")
            nc.sync.dma_start(bgv_sb, bgv_ap)

            # prefetch first attention pass's eb tiles (kt 0..5) before the
            # non-critical loads; exactly `bufs` allocations so no DMA in the
            # queue waits on a consumer (would deadlock wgT behind it).
            preb = []
            for kt in range(6):
                eb_t = biasp.tile([128, 1024], bf16, name=f"ebpre{kt}", tag="bias")
                dma_eng = nc.gpsimd if kt % 2 == 0 else nc.sync
                dma_eng.dma_start(eb_t, ebT_ap[0, kt * 128:(kt + 1) * 128, 0:1024])
                preb.append(eb_t)

            wgT_sb = [pers.tile([128, C2], bf16, name=f"wgT{e}", tag=f"wgT{e}")
                      for e in range(NE)]
            for e in range(NE):
                q = nc.sync if e % 2 == 0 else nc.gpsimd
                q.dma_start(wgT_sb[e], wgT_ap[e * 128:(e + 1) * 128, :])
            ident_sb = pers.tile([128, 128], bf16, tag="ident")
            nc.sync.dma_start(ident_sb, ident_ap)
            sel2_sb = pers.tile([2, 128], f32r, tag="sel2")
            nc.sync.dma_start(sel2_sb, sel2_ap)
            # v tiles: [128 l, 130] per k-chunk: [v_h0 | ones | v_h1 | ones]
            v_all = pers.tile([128, NKT, 130], bf16, tag="v_all")
            nc.sync.dma_start(v_all[:, :, 64:65], onescols_ap.unsqueeze(2))
            nc.sync.dma_start(v_all[:, :, 129:130], onescols_ap.unsqueeze(2))
            woT_sb = pers.tile([C2, E], bf16, tag="woT")
            nc.sync.dma_start(woT_sb, woT_ap)

            q01 = pers.tile([128, L], bf16, tag="q01")
            k01 = pers.tile([128, L], bf16, tag="k01")
            g_sb = pers.tile([128, L], bf16, tag="g")
            g_lo = pers.tile([64, L], bf16, tag="g_lo")
            ygT = pers.tile([128, L], bf16, tag="ygT")
            den_sb = [pers.tile([2, 512], f32r, name=f"den{qt}", tag=f"den{qt}")
                      for qt in range(NQ)]

            # ---------------- proj + gate (interleaved) ----------------
            # gate(lh) is emitted right after proj f=2 of that lh so the
            # sigmoids drain their psum slots long before attention needs
            # them (a sigmoid-held slot at the boundary latched HAM cold).
            vT01 = work.tile([128, L], bf16, tag="vT01")
            dests = [q01, k01, vT01]
            for lh in range(2):
                for f in range(3):
                    ps = sp.tile([128, 1024], f32, name=f"pj{f}_{lh}", tag="s")
                    for e in range(NE):
                        for ltq in range(2):
                            nc.tensor.matmul(
                                ps[:, ltq * 512:(ltq + 1) * 512],
                                wpT_sb[e][:, f * 128:(f + 1) * 128],
                                xT_sb[e][:, lh * 1024 + ltq * 512:
                                          lh * 1024 + (ltq + 1) * 512],
                                start=(e == 0), stop=(e == NE - 1))
                    if f == 2:
                        # vT drain on ScalarE (idle until attention exps)
                        nc.scalar.copy(
                            dests[f][:, lh * 1024:(lh + 1) * 1024], ps)
                    else:
                        nc.vector.tensor_copy(
                            dests[f][:, lh * 1024:(lh + 1) * 1024], ps)
                ps = sp.tile([C2, 1024], f32, name=f"pg{lh}", tag="s")
                for e in range(NE):
                    for ltq in range(2):
                        nc.tensor.matmul(
                            ps[:, ltq * 512:(ltq + 1) * 512], wgT_sb[e],
                            xT_sb[e][:, lh * 1024 + ltq * 512:
                                      lh * 1024 + (ltq + 1) * 512],
                            start=(e == 0), stop=(e == NE - 1))
                nc.scalar.activation(
                    g_sb[:, lh * 1024:(lh + 1) * 1024], ps,
                    mybir.ActivationFunctionType.Sigmoid,
                    bias=bgv_sb, scale=1.0)

            # transpose vT01 -> v_all[:, kt, :] via PE (bf16 transpose)
            for kt in range(NKT):
                ps = sp.tile([128, 128], bf16, name=f"tr{kt}", tag="s")
                nc.tensor.transpose(
                    ps, vT01[:, kt * 128:(kt + 1) * 128], ident_sb)
                nc.vector.tensor_copy(v_all[:, kt, 0:64], ps[:, 0:64])
                nc.vector.tensor_copy(v_all[:, kt, 65:129], ps[:, 64:128])

            # partition-shifted copy of g rows 64..127 for the h1 epilogue mul
            nc.sync.dma_start(g_lo, g_sb[64:128, :])

            # ---------------- attention ----------------
            # Pass over (head, q-tiles), software-pipelined 2 deep: y(kt-2)
            # is emitted after s(kt) so no PE instruction waits at the queue
            # head (queue-head waits read as idle to HAM and latch the clock
            # cold).  hooks[kt] emits the PREVIOUS pass's epilogue / tails
            # inside this pass's kt loop so the PE never goes quiet at a
            # pass boundary.  Each pass returns its epilogue as a closure.
            def attention_pass(h, qts, preb=None, hooks=None):
                hb = h * 64
                q0 = qts[0] * 512
                W = 512 * len(qts)
                y_ps = {qt: yp.tile([65, 512], f32, name=f"y{h}_{qt}",
                                    tag=f"y{qt % 2}", bufs=1) for qt in qts}
                pend = []  # [(kt, p_tile), ...] awaiting y matmuls
                def flush_y(upto):
                    while pend and pend[0][0] <= upto:
                        pkt, pt = pend.pop(0)
                        for i, qt in enumerate(qts):
                            nc.tensor.matmul(
                                y_ps[qt],
                                v_all[:, pkt, h * 65:(h + 1) * 65],
                                pt[:, i * 512:(i + 1) * 512],
                                start=(pkt == 0), stop=(pkt == NKT - 1))
                for kt in range(NKT):
                    if preb is not None and kt < len(preb):
                        eb_t = preb[kt]
                    else:
                        eb_t = biasp.tile([128, W], bf16,
                                          name=f"eb{h}_{q0}_{kt}", tag="bias")
                        dma_eng = nc.gpsimd if kt % 2 == 0 else nc.sync
                        dma_eng.dma_start(
                            eb_t, ebT_ap[h, kt * 128:(kt + 1) * 128,
                                         q0:q0 + W])
                    s_ps = sp.tile([128, W], f32,
                                   name=f"s{h}_{q0}_{kt}", tag="s")
                    for i, qt in enumerate(qts):
                        nc.tensor.matmul(
                            s_ps[:, i * 512:(i + 1) * 512],
                            k01[hb:hb + 64, kt * 128:(kt + 1) * 128],
                            q01[hb:hb + 64, qt * 512:(qt + 1) * 512],
                            start=True, stop=True)
                    flush_y(kt - 2)
                    if hooks and kt in hooks:
                        hooks[kt]()
                    eqk_t = ep.tile([128, W], bf16,
                                    name=f"eqk{h}_{q0}_{kt}", tag="eqk")
                    nc.scalar.activation(
                        eqk_t, s_ps, mybir.ActivationFunctionType.Exp)
                    p_t = pp.tile([128, W], bf16,
                                  name=f"p{h}_{q0}_{kt}", tag="p")
                    nc.vector.tensor_mul(p_t, eqk_t, eb_t)
                    pend.append((kt, p_t))
                flush_y(NKT - 1)

                def epilogue():
                    for qt in qts:
                        qsl = slice(qt * 512, (qt + 1) * 512)
                        denrow = nrm.tile([65, 512], f32r,
                                          name=f"denrow{h}_{qt}", tag="denrow")
                        nc.vector.tensor_copy(denrow[64:65, :],
                                              y_ps[qt][64:65, :])
                        nc.sync.dma_start(den_sb[qt][h:h + 1, :],
                                          denrow[64:65, :])
                        if h == 0:
                            nc.vector.tensor_mul(
                                ygT[0:64, qsl], y_ps[qt][0:64, :],
                                g_sb[0:64, qsl])
                        else:
                            yg1 = nrm.tile([64, 512], bf16,
                                           name=f"yg1_{qt}", tag="yg1")
                            nc.vector.tensor_mul(
                                yg1, y_ps[qt][0:64, :], g_lo[:, qsl])
                            nc.sync.dma_start(ygT[64:128, qsl], yg1)
                return epilogue

            def tail_rb(qt):
                # broadcast both heads' denominators and normalize ygT
                qsl = slice(qt * 512, (qt + 1) * 512)
                ps_rb = sp.tile([128, 512], f32, name=f"rb_ps{qt}", tag="s")
                nc.tensor.matmul(ps_rb, sel2_sb, den_sb[qt],
                                 start=True, stop=True)
                rbT = nrm.tile([128, 512], f32, name=f"rbT{qt}", tag="rbT")
                nc.vector.reciprocal_approx_fast(rbT, ps_rb)
                nc.vector.tensor_mul(ygT[:, qsl], ygT[:, qsl], rbT)

            def tail_oproj(qt):
                qsl = slice(qt * 512, (qt + 1) * 512)
                for eo in range(NE):
                    ps = sp.tile([128, 512], f32, name=f"po{qt}_{eo}", tag="s")
                    nc.tensor.matmul(
                        ps, woT_sb[:, eo * 128:(eo + 1) * 128], ygT[:, qsl],
                        start=True, stop=True)
                    ot = outp.tile([128, 512], f32, name=f"ot{qt}_{eo}", tag="ot")
                    # all drains on DVE, never ScalarE: a scalar copy inside
                    # an attention hook stalls the exp cadence (HAM latch);
                    # gpsimd cannot read PSUM.
                    nc.vector.tensor_copy(ot, ps)
                    nc.sync.dma_start(
                        outT_ap[eo * 128:(eo + 1) * 128, qsl], ot)

            e0 = attention_pass(0, [0, 1], preb=preb)
            e1 = attention_pass(1, [0, 1], hooks={1: e0})
            e2 = attention_pass(0, [2, 3], hooks={1: e1})
            e3 = attention_pass(1, [2, 3], hooks={
                1: e2,
                3: lambda: tail_rb(0),
                6: lambda: tail_oproj(0),
                9: lambda: tail_rb(1),
                12: lambda: tail_oproj(1),
            })
            e3()
            tail_rb(2)
            tail_oproj(2)
            tail_rb(3)
            tail_oproj(3)

    nc.compile()
    return nc


def kernel(x, mask, bias, w_proj, w_o, b_o, w_g, b_g):
    x = np.asarray(x, dtype=np.float32)
    mask = np.asarray(mask)
    bias = np.asarray(bias, dtype=np.float32)
    w_proj = np.asarray(w_proj, dtype=np.float32)
    w_o = np.asarray(w_o, dtype=np.float32)
    b_o = np.asarray(b_o, dtype=np.float32)
    w_g = np.asarray(w_g, dtype=np.float32)
    b_g = np.asarray(b_g, dtype=np.float32)

    if _compiled[0] is None:
        _compiled[0] = _build()
    nc = _compiled[0]

    bf = ml_dtypes.bfloat16
    xT = np.ascontiguousarray(x[0].T).astype(bf)           # [E, L]
    mask_add = np.where(mask[0], 0.0, MASK_NEG).astype(np.float32)  # [L]
    onescols = np.ones((128, NKT), dtype=bf)
    ident = np.eye(128, dtype=bf)
    sel2 = np.zeros((2, 128), dtype=np.float32)
    sel2[0, 0:64] = 1.0
    sel2[1, 64:128] = 1.0

    in_maps = []
    for c in range(N_CORES):
        heads = [c * HPC + i for i in range(HPC)]
        wpT = np.empty((E, 3 * C2), dtype=np.float32)
        for i, h in enumerate(heads):
            r0 = h * 3 * HW
            wpT[:, 0 * C2 + i * HW: 0 * C2 + (i + 1) * HW] = \
                w_proj[r0: r0 + HW].T * SCALE               # q, pre-scaled
            wpT[:, 1 * C2 + i * HW: 1 * C2 + (i + 1) * HW] = \
                w_proj[r0 + HW: r0 + 2 * HW].T              # k
            wpT[:, 2 * C2 + i * HW: 2 * C2 + (i + 1) * HW] = \
                w_proj[r0 + 2 * HW: r0 + 3 * HW].T          # v
        ebT = np.ascontiguousarray(
            bias[0, :, :, heads].transpose(0, 2, 1))        # [2, Lk, Lq]
        ebT += mask_add[None, :, None]
        ebT = np.exp(ebT).astype(bf)
        cols = slice(c * C2, (c + 1) * C2)
        wgT = np.ascontiguousarray(w_g[cols, :].T).astype(bf)  # [E, C2]
        bgv = np.ascontiguousarray(b_g[cols, None])         # [C2, 1]
        woT = np.ascontiguousarray(w_o[:, cols].T).astype(bf)  # [C2, E]
        in_maps.append({
            "xT": xT, "wpT": wpT.astype(bf), "ebT": ebT, "wgT": wgT,
            "bgv": bgv, "woT": woT, "onescols": onescols, "ident": ident,
            "sel2": sel2,
        })

    res = run_bass_kernel_spmd(nc, in_maps, list(range(N_CORES)))
    acc = res.results[0]["outT"].astype(np.float64)
    for c in range(1, N_CORES):
        acc += res.results[c]["outT"]
    out = acc.T.astype(np.float32) + b_o[None, :]
    return out[None]  # [B, L, E]
